# revision 3
# baseline (speedup 1.0000x reference)
"""Single-head causal self-attention on 8 trn2 NeuronCores (optimized).

B=16, T=4096, D=64 fp32. Data-parallel over batch: 2 batches per core.
Key optimizations over the original baseline:
  - scores = x A x^T with A = Wq^T Wk precomputed on host -> K projection
    eliminated (one qat projection instead of separate Q/K).
  - exp split across engines: ACT runs exact exp on ~70%% of off-diagonal
    score chunks; DVE runs a Schraudolph bit-trick exp (tensor_scalar
    f32->uint16: bits = round(184.665*s + 14208), relying on HW round-nearest
    + saturate-at-0) on the rest, bias -0.75 chosen so max exp fits fp16.
  - causal masking of diagonal chunks folded into the DVE exp via
    scalar_tensor_tensor with a {B_CONST, -30000} mask tile (saturation
    gives exact zeros above the diagonal) - no affine_select pass.
  - memset-seeded PE/ACT warmups (no DMA dependency) and one-superblock
    lookahead on the qat projection.

v3 = baseline structure + A-trick (scores = x A x^T, no K projection).

Morphs from the proven baseline kernel.py in minimal steps:
  - wq2 -> a2 = dup(A), A = Wq^T Wk (host); qt -> qat projection (1 matmul/block)
  - kt eliminated: ST lhsT reads xt chunks directly
  - optional (env flags): DVE schraudolph exp offload, masked-diag stt, bias -0.75
"""
import os
import sys

os.environ.setdefault("MYCRO_LOCAL_CACHE", "1")
sys.path.insert(0, "/opt/trn_rl_repo")

import numpy as np

import concourse.bass as bass
import concourse.tile as tile
from concourse import bacc, mybir
from concourse.bass_utils import run_bass_kernel_spmd

F32 = mybir.dt.float32
F16 = mybir.dt.float16
U16 = mybir.dt.uint16
ALU = mybir.AluOpType

V3_SCH = os.environ.get("V3_SCH", "1") == "1"   # DVE schraudolph on some chunks
V3_STT = os.environ.get("V3_STT", "1") == "1"   # masked-diag stt on DVE
V3_DIAG_ACT = os.environ.get("V3_DIAG_ACT", "0") == "1"  # diag exp on ACT + select
V3_PAT = os.environ.get("V3_PAT", "AAAADAAAAD")  # off-diag engine pattern
V3_YSB_ACT = os.environ.get("V3_YSB_ACT", "0") == "1"    # ytsb copy on ACT
V3_PSUM = os.environ.get("V3_PSUM", "0") == "1"          # dedicated yt/pj pools
V3_TPC = int(os.environ.get("V3_TPC", "2"))              # chunks per st tile
V3_STBUF = int(os.environ.get("V3_STBUF", "3"))          # st pool bufs
V3_DFIRST = os.environ.get("V3_DFIRST", "0") == "1"      # diag chunks first
V3_DSPLIT = os.environ.get("V3_DSPLIT", "0") == "1"      # split diag exp DVE/ACT

N_CORES = 8
B_LOC = 2
T = 4096
D = 64
NQ = 8
QB = 512
KB = 128
NCHUNK = T // KB

BIAS = -0.75 if (V3_SCH or V3_STT) else -8.0
A_SCH = 1024.0 / np.log(2.0)
A_COEF = float(A_SCH * 0.125)
B_CONST = float(15360.0 + A_SCH * BIAS - 44.0)
MASKVAL = -30000.0


def _build():
    nc = bacc.Bacc(None)

    xt_d = nc.declare_dram_parameter("xt16", [B_LOC, 128, T], F16, isOutput=False)
    a2_d = nc.declare_dram_parameter("a2", [128, 128], F16, isOutput=False)
    wv_d = nc.declare_dram_parameter("wv", [128, D], F16, isOutput=False)
    mask_d = nc.declare_dram_parameter("mask", [128, QB], F16, isOutput=False)
    yt1_d = nc.declare_dram_parameter("yt1", [B_LOC, NQ, D + 1, QB], F32, isOutput=True)

    with tile.TileContext(nc) as tc:
        with (
            tc.tile_pool(name="consts", bufs=1) as consts,
            tc.tile_pool(name="xt", bufs=2) as xt_p,
            tc.tile_pool(name="qt", bufs=2) as qt_p,
            tc.tile_pool(name="v1", bufs=2) as v1_p,
            tc.tile_pool(name="pt", bufs=3) as pt_p,
            tc.tile_pool(name="scratch", bufs=2) as scratch_p,
            tc.tile_pool(name="stps", bufs=(2 if V3_PSUM else V3_STBUF), space="PSUM") as st_ps,
            tc.tile_pool(name="ytps", bufs=2, space="PSUM") as yt_ps,
            tc.tile_pool(name="pjps", bufs=2, space="PSUM") as pj_ps,
        ):
            if not V3_PSUM:
                pj_ps = yt_ps
            PJTAG = "pj" if V3_PSUM else "yt1"
            # ---- constants ----
            a2 = consts.tile([128, 128], F16, tag="a2")
            nc.gpsimd.dma_start(out=a2, in_=a2_d[:, :])
            wv = consts.tile([128, D], F16, tag="wv")
            nc.gpsimd.dma_start(out=wv, in_=wv_d[:, :])
            mask = consts.tile([128, QB], F16, tag="mask")
            nc.gpsimd.dma_start(out=mask, in_=mask_d[:, :])

            nbias = consts.tile([128, 1], F32, tag="nbias")
            nc.vector.memset(nbias, BIAS)

            # ---- warmups: ACT table load + PE p-state ramp (no DMA deps) ----
            wseed = consts.tile([128, 128], F16, tag="wseed")
            nc.vector.memset(wseed, 0.25)
            wsc = scratch_p.tile([128, 128], F32, tag="wexp")
            nc.scalar.activation(out=wsc, in_=wseed, func=mybir.ActivationFunctionType.Exp, scale=0.01)
            for _ in range(24):
                wps = pj_ps.tile([128, 128], F32, tag=PJTAG, name="wps")
                nc.tensor.matmul(out=wps, lhsT=wseed, rhs=wseed, start=True, stop=True)

            state = {}
            gcnt = [0]
            PATTERN = list(V3_PAT)

            def make_prologue(b):
                xt = xt_p.tile([128, T], F16, tag="xt", name="xt")
                for dj in range(NQ):
                    nc.sync.dma_start(
                        out=xt[:, QB * dj : QB * (dj + 1)],
                        in_=xt_d[b, :, QB * dj : QB * (dj + 1)],
                    )
                qat = qt_p.tile([128, T], F16, tag="qt", name="qat")
                v1 = v1_p.tile([128, NCHUNK, D + 1], F16, tag="v1", name="v1")
                nc.vector.memset(v1[:, :, D : D + 1], 1.0)
                state[b] = (xt, qat, v1)

                def proj_q(j):
                    pq = pj_ps.tile([128, QB], F32, tag=PJTAG, name="pq")
                    hq = 64 * (j % 2)
                    nc.tensor.matmul(out=pq, lhsT=a2[hq : hq + 64, :], rhs=xt[hq : hq + 64, QB * j : QB * (j + 1)], start=True, stop=True)
                    nc.vector.tensor_copy(out=qat[:, QB * j : QB * (j + 1)], in_=pq)

                def proj_v(g):
                    pvpa = pj_ps.tile([128, QB // 2], F32, tag=PJTAG, name="pvpa")
                    pvpb = pj_ps.tile([128, QB // 2], F32, tag=PJTAG, name="pvpb")
                    for k in range(8):
                        t = 8 * g + k
                        h = 64 * (k % 2)
                        dst = pvpa if k % 2 == 0 else pvpb
                        nc.tensor.matmul(
                            out=dst[:, D * (k // 2) : D * (k // 2 + 1)],
                            lhsT=xt[h : h + 64, 128 * t : 128 * (t + 1)],
                            rhs=wv[h : h + 64, :],
                            start=True,
                            stop=True,
                        )
                    nc.vector.tensor_copy(
                        out=v1[:, 8 * g : 8 * (g + 1) : 2, 0:D],
                        in_=pvpa.rearrange("p (k c) -> p k c", c=D),
                    )
                    nc.vector.tensor_copy(
                        out=v1[:, 8 * g + 1 : 8 * (g + 1) : 2, 0:D],
                        in_=pvpb.rearrange("p (k c) -> p k c", c=D),
                    )

                return proj_q, proj_v

            TPC = V3_TPC

            def main_superblock(b, m):
                xt, qat, v1 = state[b]
                nch = 4 * m + 4
                if V3_DFIRST:
                    order = list(range(4 * m, nch)) + list(range(0, 4 * m))
                else:
                    order = list(range(nch))
                ntiles = (nch + TPC - 1) // TPC
                yt1 = yt_ps.tile([128, QB], F32, tag="yt1")
                st_tiles = []
                pt_tiles = []

                def info(pos):
                    c = order[pos]
                    j = c - 4 * m
                    qoff = 128 * j if j >= 0 else 0
                    return c, j, qoff

                def emit_st(pos):
                    ti, slot = divmod(pos, TPC)
                    if slot == 0:
                        st_tiles.append(st_ps.tile([128, QB * TPC], F32, tag="st", name="st_t"))
                    st_t = st_tiles[ti]
                    c, j, qoff = info(pos)
                    half = 64 * (c % 2)
                    nc.tensor.matmul(
                        out=st_t[:, QB * slot + qoff : QB * (slot + 1)],
                        lhsT=xt[half : half + 64, KB * c : KB * (c + 1)],
                        rhs=qat[half : half + 64, QB * m + qoff : QB * (m + 1)],
                        start=True,
                        stop=True,
                    )

                def exp_diag_stt(st_t, pt, slot, qoff):
                    w = QB - qoff
                    nc.vector.scalar_tensor_tensor(
                        out=pt[:, QB * slot + qoff : QB * (slot + 1)].bitcast(U16),
                        in0=st_t[:, QB * slot + qoff : QB * (slot + 1)],
                        scalar=A_COEF,
                        in1=mask[:, 0:w],
                        op0=ALU.mult,
                        op1=ALU.add,
                    )

                def exp_diag_act(st_t, pt, slot, qoff, j):
                    nc.scalar.activation(
                        out=pt[:, QB * slot + qoff : QB * (slot + 1)],
                        in_=st_t[:, QB * slot + qoff : QB * (slot + 1)],
                        func=mybir.ActivationFunctionType.Exp,
                        bias=nbias,
                        scale=0.125,
                    )
                    sub = pt[:, QB * slot + qoff : QB * slot + qoff + 128]
                    nc.gpsimd.affine_select(
                        out=sub,
                        in_=sub,
                        compare_op=mybir.AluOpType.is_ge,
                        fill=0.0,
                        base=0,
                        pattern=[[1, 128]],
                        channel_multiplier=-1,
                    )

                def emit_act_pv(ti):
                    st_t = st_tiles[ti]
                    p0 = TPC * ti
                    p1 = min(p0 + TPC, nch)
                    pt = pt_p.tile([128, QB * TPC], F16, tag="pt", name="pt")
                    pt_tiles.append(pt)
                    has_diag = any(info(p)[1] >= 0 for p in range(p0, p1))
                    if has_diag:
                        for p in range(p0, p1):
                            c, j, qoff = info(p)
                            slot = p - p0
                            if V3_STT and (not V3_DSPLIT or j % 2 == 0) and not V3_DIAG_ACT:
                                exp_diag_stt(st_t, pt, slot, qoff)
                            else:
                                exp_diag_act(st_t, pt, slot, qoff, j)
                    else:
                        width = QB * (p1 - p0)
                        eng = PATTERN[gcnt[0] % len(PATTERN)] if V3_SCH else "A"
                        gcnt[0] += 1
                        if eng == "A":
                            nc.scalar.activation(
                                out=pt[:, :width],
                                in_=st_t[:, :width],
                                func=mybir.ActivationFunctionType.Exp,
                                bias=nbias,
                                scale=0.125,
                            )
                        else:
                            nc.vector.tensor_scalar(
                                out=pt[:, :width].bitcast(U16),
                                in0=st_t[:, :width],
                                scalar1=A_COEF,
                                scalar2=B_CONST,
                                op0=ALU.mult,
                                op1=ALU.add,
                            )
                    for p in range(p0, p1):
                        c, j, qoff = info(p)
                        slot = p - p0
                        nc.tensor.matmul(
                            out=yt1[0 : D + 1, qoff:QB],
                            lhsT=v1[:, c, :],
                            rhs=pt[:, QB * slot + qoff : QB * (slot + 1)],
                            start=(p == 0),
                            stop=(p == nch - 1),
                            skip_group_check=True,
                        )

                for p in range(min(TPC, nch)):
                    emit_st(p)
                for ti in range(1, ntiles):
                    for p in range(TPC * ti, min(TPC * (ti + 1), nch)):
                        emit_st(p)
                    emit_act_pv(ti - 1)
                emit_act_pv(ntiles - 1)
                ytsb = scratch_p.tile([D + 1, QB], F32, tag="ytsb", name="ytsb")
                if V3_YSB_ACT:
                    nc.scalar.copy(out=ytsb, in_=yt1[0 : D + 1, :])
                else:
                    nc.vector.tensor_copy(out=ytsb, in_=yt1[0 : D + 1, :])
                nc.sync.dma_start(out=yt1_d[b, m, :, :], in_=ytsb)

            # emission: like baseline, proj interleaved between superblocks
            pq0, pv0 = make_prologue(0)
            pq0(0); pq0(1); pv0(0)
            main_superblock(0, 0)
            pq0(2); main_superblock(0, 1)
            pq0(3); pv0(1); main_superblock(0, 2)
            pq0(4); main_superblock(0, 3)
            pq0(5); pv0(2); main_superblock(0, 4)
            pq1, pv1 = make_prologue(1)
            pq0(6); pv0(3); main_superblock(0, 5)
            pq0(7); pq1(0); pq1(1); pv1(0); main_superblock(0, 6)
            pq1(2); pq1(3); pv1(1); main_superblock(0, 7)
            pq1(4); pq1(5); pv1(2); main_superblock(1, 0)
            pq1(6); pq1(7); pv1(3); main_superblock(1, 1)
            main_superblock(1, 2)
            for m in range(3, NQ):
                main_superblock(1, m)

    nc.finalize()
    return nc


_NC = None


def _get_nc():
    global _NC
    if _NC is None:
        _NC = _build()
    return _NC


def _run(x, Wk, Wq, Wv, trace=False):
    x = np.ascontiguousarray(np.asarray(x, dtype=np.float32))
    Wk = np.asarray(Wk, dtype=np.float32)
    Wq = np.asarray(Wq, dtype=np.float32)
    Wv = np.asarray(Wv, dtype=np.float32)
    B = x.shape[0]
    assert B == N_CORES * B_LOC and x.shape[1] == T and x.shape[2] == D

    A = (Wq.T @ Wk).astype(np.float32)
    a2 = np.block([[A, A], [A, A]]).astype(np.float16)
    wv2 = np.ascontiguousarray(np.concatenate([Wv.T, Wv.T], axis=0)).astype(np.float16)
    kk = np.arange(128)[:, None]
    cc = np.arange(QB)[None, :]
    mask = np.where(cc >= kk, np.float32(B_CONST), np.float32(MASKVAL)).astype(np.float16)

    xt16 = x.astype(np.float16).transpose(0, 2, 1)
    xt16 = np.ascontiguousarray(np.concatenate([xt16, xt16], axis=1))
    in_maps = []
    for c in range(N_CORES):
        in_maps.append(
            {
                "xt16": np.ascontiguousarray(xt16[B_LOC * c : B_LOC * (c + 1)]),
                "a2": a2,
                "wv": wv2,
                "mask": mask,
            }
        )

    nc = _get_nc()
    res = run_bass_kernel_spmd(nc, in_maps, core_ids=list(range(N_CORES)), trace=trace)

    y = np.empty((B, T, D), dtype=np.float32)
    for c in range(N_CORES):
        yt1 = res.results[c]["yt1"]
        num = yt1[:, :, :D, :]
        den = yt1[:, :, D : D + 1, :]
        yb = (num / den).transpose(0, 1, 3, 2).reshape(B_LOC, T, D)
        y[B_LOC * c : B_LOC * (c + 1)] = yb
    return y, res


def kernel(x, Wk, Wq, Wv):
    y, _ = _run(x, Wk, Wq, Wv, trace=False)
    return y


# revision 4
# speedup vs baseline: 1.0996x; 1.0996x over previous
"""Single-head causal self-attention on 8 trn2 NeuronCores (optimized).

B=16, T=4096, D=64 fp32. Data-parallel over batch: 2 batches per core.
Optimizations over the original baseline:
  - scores = x A x^T with A = Wq^T Wk precomputed on host (K projection gone)
  - exp split: ACT exact exp on 80%% of off-diag chunks, DVE Schraudolph
    bit-trick exp (f32->uint16 tensor_scalar, HW round-nearest + saturate)
    on the rest; causal mask folded into the DVE op via a mask tile
  - deep cross-superblock software pipeline: each superblock tail (exp/PV/
    store) interleaves with the next superblock's first ST tiles, with yt
    PSUM buffer parity forced so two accumulation groups ping-pong
  - memset-seeded PE/ACT warmups, one-superblock qat projection lookahead

v3 = baseline structure + A-trick (scores = x A x^T, no K projection).

Morphs from the proven baseline kernel.py in minimal steps:
  - wq2 -> a2 = dup(A), A = Wq^T Wk (host); qt -> qat projection (1 matmul/block)
  - kt eliminated: ST lhsT reads xt chunks directly
  - optional (env flags): DVE schraudolph exp offload, masked-diag stt, bias -0.75
"""
import os
import sys

os.environ.setdefault("MYCRO_LOCAL_CACHE", "1")
sys.path.insert(0, "/opt/trn_rl_repo")

import numpy as np

import concourse.bass as bass
import concourse.tile as tile
from concourse import bacc, mybir
from concourse.bass_utils import run_bass_kernel_spmd

F32 = mybir.dt.float32
F16 = mybir.dt.float16
U16 = mybir.dt.uint16
ALU = mybir.AluOpType

V3_SCH = os.environ.get("V3_SCH", "1") == "1"   # DVE schraudolph on some chunks
V3_STT = os.environ.get("V3_STT", "1") == "1"   # masked-diag stt on DVE
V3_DIAG_ACT = os.environ.get("V3_DIAG_ACT", "0") == "1"  # diag exp on ACT + select
V3_PAT = os.environ.get("V3_PAT", "AAAADAAAAD")  # off-diag engine pattern
V3_YSB_ACT = os.environ.get("V3_YSB_ACT", "0") == "1"    # ytsb copy on ACT
V3_PSUM = os.environ.get("V3_PSUM", "0") == "1"          # dedicated yt/pj pools
V3_TPC = int(os.environ.get("V3_TPC", "2"))              # chunks per st tile
V3_STBUF = int(os.environ.get("V3_STBUF", "3"))          # st pool bufs
V3_DFIRST = os.environ.get("V3_DFIRST", "0") == "1"      # diag chunks first
V3_DEEP = os.environ.get("V3_DEEP", "1") == "1"          # deeper cross-sb overlap
V3_DSPLIT = os.environ.get("V3_DSPLIT", "0") == "1"      # split diag exp DVE/ACT
V3_POS = float(os.environ.get("V3_POS", "0"))            # >0: DVE for first POS frac of sb off-diag tiles

N_CORES = 8
B_LOC = 2
T = 4096
D = 64
NQ = 8
QB = 512
KB = 128
NCHUNK = T // KB

BIAS = -0.75 if (V3_SCH or V3_STT) else -8.0
A_SCH = 1024.0 / np.log(2.0)
A_COEF = float(A_SCH * 0.125)
B_CONST = float(15360.0 + A_SCH * BIAS - 44.0)
MASKVAL = -30000.0


def _build():
    nc = bacc.Bacc(None)

    xt_d = nc.declare_dram_parameter("xt16", [B_LOC, 128, T], F16, isOutput=False)
    a2_d = nc.declare_dram_parameter("a2", [128, 128], F16, isOutput=False)
    wv_d = nc.declare_dram_parameter("wv", [128, D], F16, isOutput=False)
    mask_d = nc.declare_dram_parameter("mask", [128, QB], F16, isOutput=False)
    yt1_d = nc.declare_dram_parameter("yt1", [B_LOC, NQ, D + 1, QB], F32, isOutput=True)

    with tile.TileContext(nc) as tc:
        with (
            tc.tile_pool(name="consts", bufs=1) as consts,
            tc.tile_pool(name="xt", bufs=2) as xt_p,
            tc.tile_pool(name="qt", bufs=2) as qt_p,
            tc.tile_pool(name="v1", bufs=2) as v1_p,
            tc.tile_pool(name="pt", bufs=3) as pt_p,
            tc.tile_pool(name="scratch", bufs=2) as scratch_p,
            tc.tile_pool(name="stps", bufs=(2 if V3_PSUM else V3_STBUF), space="PSUM") as st_ps,
            tc.tile_pool(name="ytps", bufs=2, space="PSUM") as yt_ps,
            tc.tile_pool(name="pjps", bufs=2, space="PSUM") as pj_ps,
        ):
            if not V3_PSUM:
                pj_ps = yt_ps
            PJTAG = "pj" if V3_PSUM else "yt1"
            # ---- constants ----
            a2 = consts.tile([128, 128], F16, tag="a2")
            nc.gpsimd.dma_start(out=a2, in_=a2_d[:, :])
            wv = consts.tile([128, D], F16, tag="wv")
            nc.gpsimd.dma_start(out=wv, in_=wv_d[:, :])
            mask = consts.tile([128, QB], F16, tag="mask")
            nc.gpsimd.dma_start(out=mask, in_=mask_d[:, :])

            nbias = consts.tile([128, 1], F32, tag="nbias")
            nc.vector.memset(nbias, BIAS)

            ytparity = [0, -1]  # [count of tag-yt1 allocs, buf idx of last yt]
            # ---- warmups: ACT table load + PE p-state ramp (no DMA deps) ----
            wseed = consts.tile([128, 128], F16, tag="wseed")
            nc.vector.memset(wseed, 0.25)
            wsc = scratch_p.tile([128, 128], F32, tag="wexp")
            nc.scalar.activation(out=wsc, in_=wseed, func=mybir.ActivationFunctionType.Exp, scale=0.01)
            for _ in range(24):
                ytparity[0] += 1
                wps = pj_ps.tile([128, 128], F32, tag=PJTAG, name="wps")
                nc.tensor.matmul(out=wps, lhsT=wseed, rhs=wseed, start=True, stop=True)

            state = {}
            gcnt = [0]
            PATTERN = list(V3_PAT)

            def make_prologue(b):
                xt = xt_p.tile([128, T], F16, tag="xt", name="xt")
                for dj in range(NQ):
                    nc.sync.dma_start(
                        out=xt[:, QB * dj : QB * (dj + 1)],
                        in_=xt_d[b, :, QB * dj : QB * (dj + 1)],
                    )
                qat = qt_p.tile([128, T], F16, tag="qt", name="qat")
                v1 = v1_p.tile([128, NCHUNK, D + 1], F16, tag="v1", name="v1")
                nc.vector.memset(v1[:, :, D : D + 1], 1.0)
                state[b] = (xt, qat, v1)

                def proj_q(j):
                    ytparity[0] += 1
                    pq = pj_ps.tile([128, QB], F32, tag=PJTAG, name="pq")
                    hq = 64 * (j % 2)
                    nc.tensor.matmul(out=pq, lhsT=a2[hq : hq + 64, :], rhs=xt[hq : hq + 64, QB * j : QB * (j + 1)], start=True, stop=True)
                    nc.vector.tensor_copy(out=qat[:, QB * j : QB * (j + 1)], in_=pq)

                def proj_v(g):
                    ytparity[0] += 2
                    pvpa = pj_ps.tile([128, QB // 2], F32, tag=PJTAG, name="pvpa")
                    pvpb = pj_ps.tile([128, QB // 2], F32, tag=PJTAG, name="pvpb")
                    for k in range(8):
                        t = 8 * g + k
                        h = 64 * (k % 2)
                        dst = pvpa if k % 2 == 0 else pvpb
                        nc.tensor.matmul(
                            out=dst[:, D * (k // 2) : D * (k // 2 + 1)],
                            lhsT=xt[h : h + 64, 128 * t : 128 * (t + 1)],
                            rhs=wv[h : h + 64, :],
                            start=True,
                            stop=True,
                        )
                    nc.vector.tensor_copy(
                        out=v1[:, 8 * g : 8 * (g + 1) : 2, 0:D],
                        in_=pvpa.rearrange("p (k c) -> p k c", c=D),
                    )
                    nc.vector.tensor_copy(
                        out=v1[:, 8 * g + 1 : 8 * (g + 1) : 2, 0:D],
                        in_=pvpb.rearrange("p (k c) -> p k c", c=D),
                    )

                return proj_q, proj_v

            TPC = V3_TPC

            def main_superblock(b, m):
                xt, qat, v1 = state[b]
                nch = 4 * m + 4
                if V3_DFIRST:
                    order = list(range(4 * m, nch)) + list(range(0, 4 * m))
                else:
                    order = list(range(nch))
                ntiles = (nch + TPC - 1) // TPC
                box = {}
                st_tiles = []
                pt_tiles = []

                def info(pos):
                    c = order[pos]
                    j = c - 4 * m
                    qoff = 128 * j if j >= 0 else 0
                    return c, j, qoff

                def get_yt():
                    if "yt" not in box:
                        idx = ytparity[0] % 2
                        if idx == ytparity[1]:
                            yt_ps.tile([128, 1], F32, tag="yt1", name="ytpad")
                            ytparity[0] += 1
                            idx = 1 - idx
                        box["yt"] = yt_ps.tile([128, QB], F32, tag="yt1", name="yt1")
                        ytparity[0] += 1
                        ytparity[1] = idx
                    return box["yt"]

                def emit_st(pos):
                    ti, slot = divmod(pos, TPC)
                    if slot == 0:
                        st_tiles.append(st_ps.tile([128, QB * TPC], F32, tag="st", name="st_t"))
                    st_t = st_tiles[ti]
                    c, j, qoff = info(pos)
                    half = 64 * (c % 2)
                    nc.tensor.matmul(
                        out=st_t[:, QB * slot + qoff : QB * (slot + 1)],
                        lhsT=xt[half : half + 64, KB * c : KB * (c + 1)],
                        rhs=qat[half : half + 64, QB * m + qoff : QB * (m + 1)],
                        start=True,
                        stop=True,
                    )

                def exp_diag_stt(st_t, pt, slot, qoff):
                    w = QB - qoff
                    nc.vector.scalar_tensor_tensor(
                        out=pt[:, QB * slot + qoff : QB * (slot + 1)].bitcast(U16),
                        in0=st_t[:, QB * slot + qoff : QB * (slot + 1)],
                        scalar=A_COEF,
                        in1=mask[:, 0:w],
                        op0=ALU.mult,
                        op1=ALU.add,
                    )

                def exp_diag_act(st_t, pt, slot, qoff, j):
                    nc.scalar.activation(
                        out=pt[:, QB * slot + qoff : QB * (slot + 1)],
                        in_=st_t[:, QB * slot + qoff : QB * (slot + 1)],
                        func=mybir.ActivationFunctionType.Exp,
                        bias=nbias,
                        scale=0.125,
                    )
                    sub = pt[:, QB * slot + qoff : QB * slot + qoff + 128]
                    nc.gpsimd.affine_select(
                        out=sub,
                        in_=sub,
                        compare_op=mybir.AluOpType.is_ge,
                        fill=0.0,
                        base=0,
                        pattern=[[1, 128]],
                        channel_multiplier=-1,
                    )

                def emit_act_pv(ti, do_exp=True, do_pv=True):
                    if not do_exp:
                        pt = pt_tiles[ti]
                        st_t = st_tiles[ti]
                        p0 = TPC * ti
                        p1 = min(p0 + TPC, nch)
                        emit_pv_only(ti, p0, p1, pt)
                        return
                    st_t = st_tiles[ti]
                    p0 = TPC * ti
                    p1 = min(p0 + TPC, nch)
                    pt = pt_p.tile([128, QB * TPC], F16, tag="pt", name="pt")
                    pt_tiles.append(pt)
                    has_diag = any(info(p)[1] >= 0 for p in range(p0, p1))
                    if has_diag:
                        for p in range(p0, p1):
                            c, j, qoff = info(p)
                            slot = p - p0
                            if V3_STT and (not V3_DSPLIT or j % 2 == 0) and not V3_DIAG_ACT:
                                exp_diag_stt(st_t, pt, slot, qoff)
                            else:
                                exp_diag_act(st_t, pt, slot, qoff, j)
                    else:
                        width = QB * (p1 - p0)
                        if V3_POS > 0:
                            ndt = max(ntiles - 2, 0)
                            eng = "D" if (ndt and ti < V3_POS * ndt) else "A"
                        else:
                            eng = PATTERN[gcnt[0] % len(PATTERN)] if V3_SCH else "A"
                        gcnt[0] += 1
                        if eng == "A":
                            nc.scalar.activation(
                                out=pt[:, :width],
                                in_=st_t[:, :width],
                                func=mybir.ActivationFunctionType.Exp,
                                bias=nbias,
                                scale=0.125,
                            )
                        else:
                            nc.vector.tensor_scalar(
                                out=pt[:, :width].bitcast(U16),
                                in0=st_t[:, :width],
                                scalar1=A_COEF,
                                scalar2=B_CONST,
                                op0=ALU.mult,
                                op1=ALU.add,
                            )
                    if do_pv:
                        emit_pv_only(ti, p0, p1, pt)

                def emit_pv_only(ti, p0, p1, pt):
                    for p in range(p0, p1):
                        c, j, qoff = info(p)
                        slot = p - p0
                        nc.tensor.matmul(
                            out=get_yt()[0 : D + 1, qoff:QB],
                            lhsT=v1[:, c, :],
                            rhs=pt[:, QB * slot + qoff : QB * (slot + 1)],
                            start=(p == 0),
                            stop=(p == nch - 1),
                            skip_group_check=True,
                        )

                steps = []
                for p in range(min(TPC, nch)):
                    steps.append(lambda p=p: emit_st(p))
                for ti in range(1, ntiles):
                    def step(ti=ti):
                        for p in range(TPC * ti, min(TPC * (ti + 1), nch)):
                            emit_st(p)
                        emit_act_pv(ti - 1)
                    steps.append(step)
                if V3_DEEP:
                    steps.append(lambda: emit_act_pv(ntiles - 1, do_pv=False))
                    steps.append(lambda: emit_act_pv(ntiles - 1, do_exp=False))
                else:
                    steps.append(lambda: emit_act_pv(ntiles - 1))
                def tail1():
                    ytsb = scratch_p.tile([D + 1, QB], F32, tag="ytsb", name="ytsb")
                    if V3_YSB_ACT:
                        nc.scalar.copy(out=ytsb, in_=box["yt"][0 : D + 1, :])
                    else:
                        nc.vector.tensor_copy(out=ytsb, in_=box["yt"][0 : D + 1, :])
                    nc.sync.dma_start(out=yt1_d[b, m, :, :], in_=ytsb)
                steps.append(tail1)
                return steps

            # emission: streams with 2-step tail overlap into the next superblock
            pq0, pv0 = make_prologue(0)
            seq = []  # list of (pre_closures, stream_steps)
            pq1c = {}

            def P(*fns):
                return list(fns)

            seq.append((P(lambda: pq0(0), lambda: pq0(1), lambda: pv0(0)), (0, 0)))
            seq.append((P(lambda: pq0(2)), (0, 1)))
            seq.append((P(lambda: pq0(3), lambda: pv0(1)), (0, 2)))
            seq.append((P(lambda: pq0(4)), (0, 3)))
            seq.append((P(lambda: pq0(5), lambda: pv0(2)), (0, 4)))

            def mk_b1():
                pq1c["pq"], pq1c["pv"] = make_prologue(1)
            seq.append((P(mk_b1, lambda: pq0(6), lambda: pv0(3)), (0, 5)))
            seq.append((P(lambda: pq0(7), lambda: pq1c["pq"](0), lambda: pq1c["pq"](1), lambda: pq1c["pv"](0)), (0, 6)))
            seq.append((P(lambda: pq1c["pq"](2), lambda: pq1c["pq"](3), lambda: pq1c["pv"](1)), (0, 7)))
            seq.append((P(lambda: pq1c["pq"](4), lambda: pq1c["pq"](5), lambda: pq1c["pv"](2)), (1, 0)))
            seq.append((P(lambda: pq1c["pq"](6), lambda: pq1c["pq"](7), lambda: pq1c["pv"](3)), (1, 1)))
            for m in range(2, NQ):
                seq.append((P(), (1, m)))

            prev_tail = []
            for pre, (b, m) in seq:
                for fn in pre:
                    fn()
                steps = main_superblock(b, m)
                NT = 3 if V3_DEEP else 2
                if prev_tail:
                    k = 0
                    for t in prev_tail:
                        if k < len(steps) - NT:
                            steps[k]()
                            k += 1
                        t()
                    body = steps[k:]
                else:
                    body = steps
                for s in body[:-NT]:
                    s()
                prev_tail = body[-NT:]
            for s in prev_tail:
                s()
    nc.finalize()
    return nc


_NC = None


def _get_nc():
    global _NC
    if _NC is None:
        _NC = _build()
    return _NC


def _run(x, Wk, Wq, Wv, trace=False):
    x = np.ascontiguousarray(np.asarray(x, dtype=np.float32))
    Wk = np.asarray(Wk, dtype=np.float32)
    Wq = np.asarray(Wq, dtype=np.float32)
    Wv = np.asarray(Wv, dtype=np.float32)
    B = x.shape[0]
    assert B == N_CORES * B_LOC and x.shape[1] == T and x.shape[2] == D

    A = (Wq.T @ Wk).astype(np.float32)
    a2 = np.block([[A, A], [A, A]]).astype(np.float16)
    wv2 = np.ascontiguousarray(np.concatenate([Wv.T, Wv.T], axis=0)).astype(np.float16)
    kk = np.arange(128)[:, None]
    cc = np.arange(QB)[None, :]
    mask = np.where(cc >= kk, np.float32(B_CONST), np.float32(MASKVAL)).astype(np.float16)

    xt16 = x.astype(np.float16).transpose(0, 2, 1)
    xt16 = np.ascontiguousarray(np.concatenate([xt16, xt16], axis=1))
    in_maps = []
    for c in range(N_CORES):
        in_maps.append(
            {
                "xt16": np.ascontiguousarray(xt16[B_LOC * c : B_LOC * (c + 1)]),
                "a2": a2,
                "wv": wv2,
                "mask": mask,
            }
        )

    nc = _get_nc()
    res = run_bass_kernel_spmd(nc, in_maps, core_ids=list(range(N_CORES)), trace=trace)

    y = np.empty((B, T, D), dtype=np.float32)
    for c in range(N_CORES):
        yt1 = res.results[c]["yt1"]
        num = yt1[:, :, :D, :]
        den = yt1[:, :, D : D + 1, :]
        yb = (num / den).transpose(0, 1, 3, 2).reshape(B_LOC, T, D)
        y[B_LOC * c : B_LOC * (c + 1)] = yb
    return y, res


def kernel(x, Wk, Wq, Wv):
    y, _ = _run(x, Wk, Wq, Wv, trace=False)
    return y
